# revision 3
# baseline (speedup 1.0000x reference)
"""Trainium2 Bass kernel for nn_DendriticCompartment (dense_mlp).

reference math:
    gates = sigmoid(x @ W_gate.T + b_gate)                      # (B, 4)
    seg_s = x @ W_seg[s].T + b_seg[s]                           # (B, 2048) per s
    plateau_s = sigmoid(5*(seg_s - thr_s))
    stacked_s = seg_s * plateau_s * gates[:, s:s+1]
    out = sum_s stacked_s + 0.1*sign(prod_s stacked_s)*prod_s|stacked_s|^(1/4)

Sharding: tensor-parallel on d_state (2048 -> 256 per core), every core sees the
full batch.  All 4 segments of a given d column live on one core, so the
cross-segment combine is local.  Matmuls run as float32r (TF32-like, 1 cyc/row).

Per-core layout: PSUM acc tile [128, 1024] with column j = db*512 + s*128 + dd
(db = d-block 0/1 within the 256-col slice, s = segment, dd = d within block).
"""
import numpy as np
from contextlib import ExitStack

import concourse.bass as bass
import concourse.mybir as mybir
import concourse.tile as tile
from concourse import bacc
from concourse.bass_utils import run_bass_kernel_spmd

FP32 = mybir.dt.float32
FP32R = mybir.dt.float32r

B, D_IN, D_STATE, NSEG = 8192, 2048, 2048, 4
NCORES = 8
DSLICE = D_STATE // NCORES          # 256 output cols per core
NDB = DSLICE // 128                 # 2 d-blocks per core
NL = NSEG * DSLICE                  # 1024 matmul cols per core
P = 128
KT = D_IN // P                      # 16 contraction tiles
NBT = B // P                        # 64 batch tiles


def build_nc(trace_friendly=False):
    nc = bacc.Bacc("TRN2", debug=False, target_bir_lowering=False,
                   num_devices=NCORES)

    x_d = nc.dram_tensor("x", [B, D_IN], FP32, kind="ExternalInput").ap()
    w_d = nc.dram_tensor("w", [NSEG * NDB, P, D_IN], FP32, kind="ExternalInput").ap()
    wgt_d = nc.dram_tensor("wgt", [D_IN, NSEG], FP32, kind="ExternalInput").ap()
    b1_d = nc.dram_tensor("b1", [NL], FP32, kind="ExternalInput").ap()
    d1_d = nc.dram_tensor("d1", [NL], FP32, kind="ExternalInput").ap()
    bg_d = nc.dram_tensor("bg", [NSEG], FP32, kind="ExternalInput").ap()
    id_d = nc.dram_tensor("ident", [P, P], FP32, kind="ExternalInput").ap()
    out_d = nc.dram_tensor("out", [B, DSLICE], FP32, kind="ExternalOutput").ap()

    def bcast(ap, parts=P):
        # replicate a DRAM vector across partitions (partition step 0)
        return bass.AP(tensor=ap.tensor, offset=ap.offset,
                       ap=[[0, parts]] + list(ap.ap))

    with tile.TileContext(nc) as tc, ExitStack() as ctx:
        const = ctx.enter_context(tc.tile_pool(name="const", bufs=1))
        wnat_p = ctx.enter_context(tc.tile_pool(name="wnat", bufs=2))
        xnat_p = ctx.enter_context(tc.tile_pool(name="xnat", bufs=3))
        xt_p = ctx.enter_context(tc.tile_pool(name="xt", bufs=2))
        ep_big = ctx.enter_context(tc.tile_pool(name="ep_big", bufs=2))
        ep_sm = ctx.enter_context(tc.tile_pool(name="ep_sm", bufs=2))
        out_p = ctx.enter_context(tc.tile_pool(name="out", bufs=3))
        ps_t = ctx.enter_context(tc.tile_pool(name="ps_t", bufs=2, space="PSUM"))
        ps_acc = ctx.enter_context(tc.tile_pool(name="ps_acc", bufs=2, space="PSUM"))
        ps_g = ctx.enter_context(tc.tile_pool(name="ps_g", bufs=2, space="PSUM"))

        identr = const.tile([P, P], FP32R)
        nc.sync.dma_start(out=identr[:], in_=id_d.bitcast(FP32R))

        wgt = const.tile([P, KT, NSEG], FP32R)
        nc.sync.dma_start(
            out=wgt[:], in_=wgt_d.rearrange("(kt p) s -> p kt s", p=P).bitcast(FP32R))

        b1t = const.tile([P, NL], FP32)
        d1t = const.tile([P, NL], FP32)
        bgt = const.tile([P, NSEG], FP32)
        nc.gpsimd.dma_start(out=b1t[:], in_=bcast(b1_d))
        nc.gpsimd.dma_start(out=d1t[:], in_=bcast(d1_d))
        nc.gpsimd.dma_start(out=bgt[:], in_=bcast(bg_d))

        # ---- W prep: transpose the (4, 256, 2048) slice into WT[kt][i, col] ----
        # col = db*512 + s*128 + dd
        wt = const.tile([P, KT, NL], FP32R)        # 8 MiB resident
        for s in range(NSEG):
            for db in range(NDB):
                wn = wnat_p.tile([P, D_IN], FP32R, tag="wn")
                nc.sync.dma_start(out=wn[:], in_=w_d[s * NDB + db].bitcast(FP32R))
                col0 = db * 512 + s * 128
                for ktg in range(KT // 4):
                    pt = ps_t.tile([P, 512], FP32R, tag="pt")
                    for j in range(4):
                        kt = ktg * 4 + j
                        nc.tensor.transpose(
                            pt[:, j * 128:(j + 1) * 128],
                            wn[:, kt * 128:(kt + 1) * 128], identr[:])
                    for j in range(4):
                        kt = ktg * 4 + j
                        nc.scalar.copy(
                            out=wt[:, kt, col0:col0 + 128],
                            in_=pt[:, j * 128:(j + 1) * 128])

        # ---- main loop over batch tiles ----
        x_r = x_d.rearrange("(bt p) i -> bt p i", p=P)
        out_r = out_d.rearrange("(bt p) d -> bt p d", p=P)
        for bt in range(NBT):
            xn = xnat_p.tile([P, D_IN], FP32R, tag="xn")
            nc.sync.dma_start(out=xn[:], in_=x_r[bt].bitcast(FP32R))

            xt = xt_p.tile([P, KT, P], FP32R, tag="xt")
            for ktg in range(KT // 4):
                pt = ps_t.tile([P, 512], FP32R, tag="pt")
                for j in range(4):
                    kt = ktg * 4 + j
                    nc.tensor.transpose(
                        pt[:, j * 128:(j + 1) * 128],
                        xn[:, kt * 128:(kt + 1) * 128], identr[:])
                nc.scalar.copy(
                    out=xt[:, ktg * 4:(ktg + 1) * 4, :].rearrange("p a b -> p (a b)"),
                    in_=pt[:])

            acc = ps_acc.tile([P, NL], FP32, tag="acc")       # 2 banks
            gacc = ps_g.tile([P, NSEG], FP32, tag="gacc")
            for kt in range(KT):
                nc.tensor.matmul(acc[:, 0:512], xt[:, kt, :], wt[:, kt, 0:512],
                                 start=(kt == 0), stop=(kt == KT - 1))
                nc.tensor.matmul(acc[:, 512:1024], xt[:, kt, :], wt[:, kt, 512:1024],
                                 start=(kt == 0), stop=(kt == KT - 1))
                nc.tensor.matmul(gacc[:], xt[:, kt, :], wgt[:, kt, :],
                                 start=(kt == 0), stop=(kt == KT - 1))

            # gates = sigmoid(gacc + bg)
            gsum = ep_sm.tile([P, NSEG], FP32, tag="gsum")
            nc.vector.tensor_add(gsum[:], gacc[:], bgt[:])
            gt = ep_sm.tile([P, NSEG], FP32, tag="gt")
            nc.scalar.activation(gt[:], gsum[:], mybir.ActivationFunctionType.Sigmoid)

            # epilogue over [128, 1024]
            segp = ep_big.tile([P, NL], FP32, tag="segp")     # seg + b
            parg = ep_big.tile([P, NL], FP32, tag="parg")     # seg + b - thr
            nc.vector.tensor_add(segp[:], acc[:], b1t[:])
            nc.vector.tensor_add(parg[:], acc[:], d1t[:])
            pl = ep_big.tile([P, NL], FP32, tag="pl")
            nc.scalar.activation(pl[:], parg[:],
                                 mybir.ActivationFunctionType.Sigmoid, scale=5.0)

            st = ep_big.tile([P, NDB, NSEG, P], FP32, tag="st")
            segp_v = segp[:].rearrange("p (db s dd) -> p db s dd", db=NDB, s=NSEG)
            pl_v = pl[:].rearrange("p (db s dd) -> p db s dd", db=NDB, s=NSEG)
            for s in range(NSEG):
                nc.vector.scalar_tensor_tensor(
                    out=st[:, :, s, :], in0=segp_v[:, :, s, :],
                    scalar=gt[:, s:s + 1], in1=pl_v[:, :, s, :],
                    op0=mybir.AluOpType.mult, op1=mybir.AluOpType.mult)

            sm = ep_sm.tile([P, NDB, P], FP32, tag="sm")
            pr = ep_sm.tile([P, NDB, P], FP32, tag="pr")
            nc.vector.tensor_add(sm[:], st[:, :, 0, :], st[:, :, 1, :])
            nc.vector.tensor_add(sm[:], sm[:], st[:, :, 2, :])
            nc.vector.tensor_add(sm[:], sm[:], st[:, :, 3, :])
            nc.vector.tensor_mul(pr[:], st[:, :, 0, :], st[:, :, 1, :])
            nc.vector.tensor_mul(pr[:], pr[:], st[:, :, 2, :])
            nc.vector.tensor_mul(pr[:], pr[:], st[:, :, 3, :])

            prf = pr[:].rearrange("p a b -> p (a b)")
            sgn = ep_sm.tile([P, NDB * P], FP32, tag="sgn")
            nc.scalar.sign(sgn[:], prf)
            ab = ep_sm.tile([P, NDB * P], FP32, tag="ab")
            nc.vector.tensor_mul(ab[:], prf, sgn[:])
            nc.scalar.sqrt(ab[:], ab[:])
            nc.scalar.sqrt(ab[:], ab[:])
            res = ep_sm.tile([P, NDB * P], FP32, tag="res")
            nc.vector.scalar_tensor_tensor(
                out=res[:], in0=ab[:], scalar=0.1, in1=sgn[:],
                op0=mybir.AluOpType.mult, op1=mybir.AluOpType.mult)
            ot = out_p.tile([P, NDB * P], FP32, tag="ot")
            nc.vector.tensor_add(ot[:], sm[:].rearrange("p a b -> p (a b)"), res[:])
            nc.sync.dma_start(out=out_r[bt], in_=ot[:])

    nc.compile()
    return nc


_NC_CACHE = {}


def _get_nc():
    if "nc" not in _NC_CACHE:
        _NC_CACHE["nc"] = build_nc()
    return _NC_CACHE["nc"]


def make_in_maps(x, W_seg, b_seg, threshold, W_gate, b_gate):
    x = np.ascontiguousarray(x, dtype=np.float32)
    in_maps = []
    ident = np.eye(P, dtype=np.float32)
    wgt = np.ascontiguousarray(W_gate.T, dtype=np.float32)
    for c in range(NCORES):
        sl = slice(c * DSLICE, (c + 1) * DSLICE)
        w_loc = np.ascontiguousarray(W_seg[:, sl, :], dtype=np.float32)
        w_loc = w_loc.reshape(NSEG * NDB, P, D_IN)
        b_loc = np.asarray(b_seg[:, sl], dtype=np.float32)      # (4, 256)
        t_loc = np.asarray(threshold[:, sl], dtype=np.float32)
        # col order db*512 + s*128 + dd  ->  index [db, s, dd]
        b1 = np.ascontiguousarray(
            b_loc.reshape(NSEG, NDB, P).transpose(1, 0, 2)).reshape(NL)
        d1 = np.ascontiguousarray(
            (b_loc - t_loc).reshape(NSEG, NDB, P).transpose(1, 0, 2)).reshape(NL)
        in_maps.append({
            "x": x,
            "w": w_loc,
            "wgt": wgt,
            "b1": b1,
            "d1": d1,
            "bg": np.asarray(b_gate, dtype=np.float32),
            "ident": ident,
        })
    return in_maps


def kernel(x, W_seg, b_seg, threshold, W_gate, b_gate, _trace=False, _trace_kwargs=None):
    nc = _get_nc()
    in_maps = make_in_maps(x, W_seg, b_seg, threshold, W_gate, b_gate)
    res = run_bass_kernel_spmd(nc, in_maps, core_ids=list(range(NCORES)),
                               trace=_trace, **(_trace_kwargs or {}))
    out = np.concatenate([res.results[c]["out"] for c in range(NCORES)], axis=1)
    if _trace:
        kernel.last_exec_time_ns = res.exec_time_ns
        kernel.last_results = res
    return out
